# revision 1
# baseline (speedup 1.0000x reference)
"""Trainium2 Bass kernel for a custom LSTM cell.

Math (per reference):
    i = sigmoid(x @ W_i.T + b_Wi + h @ U_i.T + b_Ui)
    f = sigmoid(x @ W_f.T + b_Wf + h @ U_f.T + b_Uf + boundary @ W_b.T + b_Wb)
    o = sigmoid(x @ W_o.T + b_Wo + h @ U_o.T + b_Uo)
    g = tanh   (x @ W_g.T + b_Wg + h @ U_g.T + b_Ug)
    c = f * c_prev + i * g
    h = o * tanh(c)

Strategy: data-parallel over batch across 8 NeuronCores (1024 rows each).
Host-side we build A.T = [x | h_prev].T (K=1536 on partitions) and a single
fused weight matrix M [1536, 4096] whose columns are ordered per 256-wide
h-slice as [i | f | o | g], so the device only does natural-layout DMAs and
K-partition matmuls. Bias + boundary enter as one extra K=3 matmul step
(lhsT rows = [ones, boundary0, boundary1]). Matmuls run in float32r (TF32)
at full PE rate; operands are rounded to f32r by DVE copies after fast
HWDGE loads.
"""

import sys

sys.path.insert(0, "/opt/trn_rl_repo")

import numpy as np

B, IN, H = 8192, 512, 1024
NCORES = 8
BLOC = B // NCORES  # 1024 rows per core
KTOT = IN + H  # 1536 contraction
KT = KTOT // 128  # 12 k-tiles
BT = BLOC // 128  # 8 batch tiles per core
SLICE = 256  # h-slice width per gate
NS = H // SLICE  # 4 slices
GW = 4 * SLICE  # 1024 columns of M per slice (i|f|o|g)

_PROG = None  # cached so repeat calls skip rebuild/recompile


def _build_program():
    import concourse.bass as bass
    import concourse.mybir as mybir
    import concourse.tile as tile
    from concourse import bacc
    from contextlib import ExitStack

    f32 = mybir.dt.float32
    f32r = mybir.dt.float32r
    bf16 = mybir.dt.bfloat16
    SIG = mybir.ActivationFunctionType.Sigmoid
    TANH = mybir.ActivationFunctionType.Tanh

    nc = bacc.Bacc("TRN2", target_bir_lowering=False, debug=False)

    at_d = nc.dram_tensor("at_in", [KTOT, BLOC], f32r, kind="ExternalInput").ap()
    et_d = nc.dram_tensor("et_in", [3, BLOC], f32r, kind="ExternalInput").ap()
    m_d = nc.dram_tensor("m_in", [KTOT, 4 * H], f32r, kind="ExternalInput").ap()
    r_d = nc.dram_tensor("r_in", [3, 4 * H], f32r, kind="ExternalInput").ap()
    c_d = nc.dram_tensor("c_in", [BLOC, H], f32, kind="ExternalInput").ap()
    h_o = nc.dram_tensor("h_out", [BLOC, H], f32, kind="ExternalOutput").ap()
    c_o = nc.dram_tensor("c_out", [BLOC, H], f32, kind="ExternalOutput").ap()

    with tile.TileContext(nc) as tc:
        with ExitStack() as ctx:
            atp = ctx.enter_context(tc.tile_pool(name="atp", bufs=1))
            mp = ctx.enter_context(tc.tile_pool(name="mp", bufs=2))
            cst = ctx.enter_context(tc.tile_pool(name="cst", bufs=1))
            cinp = ctx.enter_context(tc.tile_pool(name="cinp", bufs=4))
            actp = ctx.enter_context(tc.tile_pool(name="actp", bufs=2))
            outp = ctx.enter_context(tc.tile_pool(name="outp", bufs=4))
            psp = ctx.enter_context(tc.tile_pool(name="psp", bufs=8, space="PSUM"))
            wup = ctx.enter_context(tc.tile_pool(name="wup", bufs=1))

            # PE warm-up: dummy bf16 matmuls with no DMA deps keep the PE HAM
            # clock gate busy while the first weight tiles load.
            wu_w = wup.tile([128, 128], bf16, name="wu_w")
            nc.vector.memset(wu_w, 0.0)
            wu_ps = psp.tile([128, 512], f32, name="wu_ps", tag="ps")
            for _ in range(72):
                nc.tensor.matmul(wu_ps[:, 0:128], wu_w, wu_w, start=True, stop=True)

            et_t = cst.tile([3, BLOC], f32r, name="et_t")
            nc.sync.dma_start(out=et_t, in_=et_d[:, :])
            r_t = cst.tile([3, 4 * H], f32r, name="r_t")
            nc.sync.dma_start(out=r_t, in_=r_d[:, :])

            def load_m_slice(s):
                """One [128, 12, GW] tile per slice, filled by 3 big 3D DMAs."""
                t = mp.tile([128, KT, GW], f32r, name=f"m_{s}", tag="m")
                for j in range(3):
                    nc.sync.dma_start(
                        out=t[:, j * 4 : (j + 1) * 4, :],
                        in_=m_d[
                            j * 512 : (j + 1) * 512, s * GW : (s + 1) * GW
                        ].rearrange("(kk p) g -> p kk g", p=128),
                    )
                return t

            # AT interleaved with slice-0 weights so matmuls start early
            at_t = atp.tile([128, KT, BLOC], f32r, name="at_t")
            m_t = mp.tile([128, KT, GW], f32r, name="m_0", tag="m")
            for j in range(3):
                nc.sync.dma_start(
                    out=at_t[:, j * 4 : (j + 1) * 4, :],
                    in_=at_d[j * 512 : (j + 1) * 512, :].rearrange(
                        "(kk p) g -> p kk g", p=128
                    ),
                )
                nc.sync.dma_start(
                    out=m_t[:, j * 4 : (j + 1) * 4, :],
                    in_=m_d[j * 512 : (j + 1) * 512, 0:GW].rearrange(
                        "(kk p) g -> p kk g", p=128
                    ),
                )

            for s in range(NS):
                if s > 0:
                    m_t = load_m_slice(s)

                for b in range(BT):
                    bs = slice(b * 128, (b + 1) * 128)
                    ps_if = psp.tile([128, 512], f32, name=f"psif{s}_{b}", tag="ps")
                    ps_og = psp.tile([128, 512], f32, name=f"psog{s}_{b}", tag="ps")
                    for k in range(KT):
                        lhs = at_t[:, k, bs]
                        nc.tensor.matmul(
                            ps_if,
                            lhs,
                            m_t[:, k, 0:512],
                            start=(k == 0),
                            stop=False,
                        )
                        nc.tensor.matmul(
                            ps_og,
                            lhs,
                            m_t[:, k, 512:1024],
                            start=(k == 0),
                            stop=False,
                        )
                    # bias + boundary: K=3 step, rows [ones, bdry0, bdry1]
                    elhs = et_t[:, bs]
                    nc.tensor.matmul(
                        ps_if,
                        elhs,
                        r_t[:, s * GW : s * GW + 512],
                        start=False,
                        stop=True,
                    )
                    nc.tensor.matmul(
                        ps_og,
                        elhs,
                        r_t[:, s * GW + 512 : (s + 1) * GW],
                        start=False,
                        stop=True,
                    )

                    # gate nonlinearities (i,f -> sigmoid; o -> sigmoid; g -> tanh)
                    if_t = actp.tile([128, 512], f32, name=f"if{s}_{b}", tag="if")
                    og_t = actp.tile([128, 512], f32, name=f"og{s}_{b}", tag="og")
                    nc.scalar.activation(if_t, ps_if, SIG)
                    nc.scalar.activation(og_t[:, 0:SLICE], ps_og[:, 0:SLICE], SIG)
                    nc.scalar.activation(og_t[:, SLICE:512], ps_og[:, SLICE:512], TANH)

                    c_t = cinp.tile([128, SLICE], f32, name=f"cin{s}_{b}", tag="cin")
                    nc.scalar.dma_start(
                        out=c_t, in_=c_d[bs, s * SLICE : (s + 1) * SLICE]
                    )

                    cn = outp.tile([128, SLICE], f32, name=f"cn{s}_{b}", tag="cn")
                    tmp = actp.tile([128, SLICE], f32, name=f"tmp{s}_{b}", tag="tmp")
                    # c' = f*c_prev + i*g
                    nc.vector.tensor_mul(cn, if_t[:, SLICE:512], c_t)
                    nc.vector.tensor_mul(tmp, if_t[:, 0:SLICE], og_t[:, SLICE:512])
                    nc.vector.tensor_add(cn, cn, tmp)
                    th = actp.tile([128, SLICE], f32, name=f"th{s}_{b}", tag="th")
                    nc.scalar.activation(th, cn, TANH)
                    hn = outp.tile([128, SLICE], f32, name=f"hn{s}_{b}", tag="hn")
                    nc.vector.tensor_mul(hn, og_t[:, 0:SLICE], th)

                    nc.scalar.dma_start(
                        out=c_o[bs, s * SLICE : (s + 1) * SLICE], in_=cn
                    )
                    nc.scalar.dma_start(
                        out=h_o[bs, s * SLICE : (s + 1) * SLICE], in_=hn
                    )
    nc.compile()
    return nc


def _get_program():
    global _PROG
    if _PROG is None:
        _PROG = _build_program()
    return _PROG


def _tf32(a):
    """Round float32 ndarray to TF32 (10-bit mantissa, RNE)."""
    b = np.ascontiguousarray(a, np.float32).view(np.uint32)
    lsb = (b >> np.uint32(13)) & np.uint32(1)
    r = (b + np.uint32(0x0FFF) + lsb) & ~np.uint32(0x1FFF)
    return r.view(np.float32)


def _prep_inputs(inputs):
    """Host-side marshalling: fused weight matrix + transposed activations."""
    f = np.float32
    x = np.asarray(inputs["x"], f)
    h_prev = np.asarray(inputs["h_prev"], f)
    c_prev = np.asarray(inputs["c_prev"], f)
    boundary = np.asarray(inputs["boundary"], f)

    gates = ["i", "f", "o", "g"]
    W = {z: np.asarray(inputs[f"W_{z}"], f) for z in gates}
    U = {z: np.asarray(inputs[f"U_{z}"], f) for z in gates}
    bias = {
        z: np.asarray(inputs[f"b_W{z}"], f) + np.asarray(inputs[f"b_U{z}"], f)
        for z in gates
    }
    W_b = np.asarray(inputs["W_b"], f)
    b_Wb = np.asarray(inputs["b_Wb"], f)

    # M [1536, 4096]: rows 0-511 W.T, rows 512-1535 U.T; columns ordered per
    # 256-wide h-slice as [i | f | o | g].
    M = np.empty((KTOT, 4 * H), f)
    R = np.zeros((3, 4 * H), f)  # row0 bias; rows 1-2 boundary weights (f only)
    for s in range(NS):
        hs = slice(s * SLICE, (s + 1) * SLICE)
        for zi, z in enumerate(gates):
            cs = slice(s * GW + zi * SLICE, s * GW + (zi + 1) * SLICE)
            M[:IN, cs] = W[z][hs].T
            M[IN:, cs] = U[z][hs].T
            R[0, cs] = bias[z][hs]
            if z == "f":
                R[0, cs] += b_Wb[hs]
                R[1:3, cs] = W_b[hs].T

    AT = np.concatenate([x, h_prev], axis=1).T  # [1536, 8192]
    ET = np.concatenate(
        [np.ones((1, B), f), boundary.T.astype(f)], axis=0
    )  # [3, 8192]

    MR = _tf32(M)
    RR = _tf32(R)
    in_maps = []
    for c in range(NCORES):
        rs = slice(c * BLOC, (c + 1) * BLOC)
        in_maps.append(
            {
                "at_in": _tf32(AT[:, rs]),
                "et_in": _tf32(ET[:, rs]),
                "m_in": MR,
                "r_in": RR,
                "c_in": np.ascontiguousarray(c_prev[rs]),
            }
        )
    return in_maps


def run(inputs, trace=False):
    """Returns ((h, c), BassKernelResults)."""
    from concourse.bass_utils import run_bass_kernel_spmd

    nc = _get_program()
    in_maps = _prep_inputs(inputs)
    res = run_bass_kernel_spmd(
        nc, in_maps, core_ids=list(range(NCORES)), trace=trace
    )
    h = np.concatenate([r["h_out"] for r in res.results], axis=0)
    c = np.concatenate([r["c_out"] for r in res.results], axis=0)
    return (h, c), res


def kernel(**inputs):
    out, _ = run(inputs, trace=False)
    return out



# revision 2
# speedup vs baseline: 236.9991x; 236.9991x over previous
"""Trainium2 Bass kernel for a custom LSTM cell.

Math (per reference):
    i = sigmoid(x @ W_i.T + b_Wi + h @ U_i.T + b_Ui)
    f = sigmoid(x @ W_f.T + b_Wf + h @ U_f.T + b_Uf + boundary @ W_b.T + b_Wb)
    o = sigmoid(x @ W_o.T + b_Wo + h @ U_o.T + b_Uo)
    g = tanh   (x @ W_g.T + b_Wg + h @ U_g.T + b_Ug)
    c = f * c_prev + i * g
    h = o * tanh(c)

Strategy: data-parallel over batch across 8 NeuronCores (1024 rows each),
computed TRANSPOSED on-device: hidden on partitions, batch on the free axis.
With hidden on partitions the gate biases become per-partition ACT-engine
bias operands (free), and the boundary term is a K=2 matmul accumulated
straight into the f-gate PSUM group — no K=3 bias matmuls.

Matmul operands are bf16 (well within the 2e-2 error budget), halving HBM
traffic vs f32/f32r. Per h-slice of 128 hidden rows the gates run in two
waves (i,g then f,o) of [128,512] PSUM tiles so the 8 PSUM banks hold two
(slice, batch-half) units in flight and the PE never waits on drains.
"""

import sys

sys.path.insert(0, "/opt/trn_rl_repo")

import numpy as np
import ml_dtypes

BF16 = ml_dtypes.bfloat16

B, IN, H = 8192, 512, 1024
NCORES = 8
BLOC = B // NCORES  # 1024 batch rows per core
KTOT = IN + H  # 1536 contraction
KT = KTOT // 128  # 12 k-tiles
NS = H // 128  # 8 h-slices of 128 hidden rows
GW = 4 * 128  # 512 columns of M per h-slice (i|g|f|o)
HALF = BLOC // 2  # 512-wide batch halves (one PSUM bank each)

_PROG = None  # cached so repeat calls skip rebuild/recompile


def _build_program():
    import concourse.mybir as mybir
    import concourse.tile as tile
    from concourse import bacc
    from contextlib import ExitStack

    f32 = mybir.dt.float32
    bf = mybir.dt.bfloat16
    SIG = mybir.ActivationFunctionType.Sigmoid
    TANH = mybir.ActivationFunctionType.Tanh

    nc = bacc.Bacc("TRN2", target_bir_lowering=False, debug=False)

    a_d = nc.dram_tensor("a_in", [KTOT, BLOC], bf, kind="ExternalInput").ap()
    m_d = nc.dram_tensor("m_in", [KTOT, 4 * H], bf, kind="ExternalInput").ap()
    wb_d = nc.dram_tensor("wb_in", [2, H], bf, kind="ExternalInput").ap()
    bd_d = nc.dram_tensor("bd_in", [2, BLOC], bf, kind="ExternalInput").ap()
    bias_d = nc.dram_tensor("bias_in", [128, 4 * NS], f32, kind="ExternalInput").ap()
    ct_d = nc.dram_tensor("ct_in", [H, BLOC], f32, kind="ExternalInput").ap()
    ht_o = nc.dram_tensor("ht_out", [H, BLOC], f32, kind="ExternalOutput").ap()
    ct_o = nc.dram_tensor("ct_out", [H, BLOC], f32, kind="ExternalOutput").ap()

    with tile.TileContext(nc) as tc:
        with ExitStack() as ctx:
            apl = ctx.enter_context(tc.tile_pool(name="apl", bufs=1))
            mp = ctx.enter_context(tc.tile_pool(name="mp", bufs=3))
            cst = ctx.enter_context(tc.tile_pool(name="cst", bufs=1))
            ctp = ctx.enter_context(tc.tile_pool(name="ctp", bufs=2))
            gp = ctx.enter_context(tc.tile_pool(name="gp", bufs=6))
            ep = ctx.enter_context(tc.tile_pool(name="ep", bufs=4))
            outp = ctx.enter_context(tc.tile_pool(name="outp", bufs=4))
            psp = ctx.enter_context(tc.tile_pool(name="psp", bufs=8, space="PSUM"))
            wup = ctx.enter_context(tc.tile_pool(name="wup", bufs=1))

            # PE warm-up: dummy bf16 matmuls with no DMA deps ramp the PE
            # p-state while the first weight tiles load.
            wu_w = wup.tile([128, 128], bf, name="wu_w")
            nc.vector.memset(wu_w, 0.0)
            wu_ps = psp.tile([128, 512], f32, name="wu_ps", tag="ps")
            for _ in range(32):
                nc.tensor.matmul(wu_ps[:, 0:128], wu_w, wu_w, start=True, stop=True)

            bias_t = cst.tile([128, 4 * NS], f32, name="bias_t")
            nc.scalar.dma_start(out=bias_t, in_=bias_d[:, :])
            wb_t = cst.tile([2, H], bf, name="wb_t")
            nc.scalar.dma_start(out=wb_t, in_=wb_d[:, :])
            bd_t = cst.tile([2, BLOC], bf, name="bd_t")
            nc.scalar.dma_start(out=bd_t, in_=bd_d[:, :])

            def load_m_slice(s):
                """[128, 12, 512] weight tile for h-slice s, 3 big 3D DMAs."""
                t = mp.tile([128, KT, GW], bf, name=f"m_{s}", tag="m")
                for j in range(3):
                    nc.sync.dma_start(
                        out=t[:, j * 4 : (j + 1) * 4, :],
                        in_=m_d[
                            j * 512 : (j + 1) * 512, s * GW : (s + 1) * GW
                        ].rearrange("(kk p) g -> p kk g", p=128),
                    )
                return t

            def load_ct_slice(s):
                t = ctp.tile([128, BLOC], f32, name=f"ct_{s}", tag="ct")
                nc.scalar.dma_start(out=t, in_=ct_d[s * 128 : (s + 1) * 128, :])
                return t

            # A interleaved with slice-0 weights so matmuls start early
            a_t = apl.tile([128, KT, BLOC], bf, name="a_t")
            m_t = mp.tile([128, KT, GW], bf, name="m_0", tag="m")
            for j in range(3):
                nc.sync.dma_start(
                    out=m_t[:, j * 4 : (j + 1) * 4, :],
                    in_=m_d[j * 512 : (j + 1) * 512, 0:GW].rearrange(
                        "(kk p) g -> p kk g", p=128
                    ),
                )
                nc.sync.dma_start(
                    out=a_t[:, j * 4 : (j + 1) * 4, :],
                    in_=a_d[j * 512 : (j + 1) * 512, :].rearrange(
                        "(kk p) b -> p kk b", p=128
                    ),
                )
            ct_t = load_ct_slice(0)

            for s in range(NS):
                if s > 0:
                    m_t = load_m_slice(s)
                    ct_t = load_ct_slice(s)
                b0 = 4 * s
                for h2 in range(2):
                    bs = slice(h2 * HALF, (h2 + 1) * HALF)
                    # wave 1: i, g
                    ps_i = psp.tile([128, HALF], f32, name=f"psi{s}_{h2}", tag="ps")
                    ps_g = psp.tile([128, HALF], f32, name=f"psg{s}_{h2}", tag="ps")
                    for k in range(KT):
                        rhs = a_t[:, k, bs]
                        nc.tensor.matmul(
                            ps_i, m_t[:, k, 0:128], rhs,
                            start=(k == 0), stop=(k == KT - 1),
                        )
                        nc.tensor.matmul(
                            ps_g, m_t[:, k, 128:256], rhs,
                            start=(k == 0), stop=(k == KT - 1),
                        )
                    i_t = gp.tile([128, HALF], f32, name=f"i{s}_{h2}", tag="g")
                    g_t = gp.tile([128, HALF], f32, name=f"g{s}_{h2}", tag="g")
                    nc.scalar.activation(i_t, ps_i, SIG, bias=bias_t[:, b0 : b0 + 1])
                    nc.scalar.activation(
                        g_t, ps_g, TANH, bias=bias_t[:, b0 + 1 : b0 + 2]
                    )
                    ig_t = ep.tile([128, HALF], f32, name=f"ig{s}_{h2}", tag="ig")
                    nc.vector.tensor_mul(ig_t, i_t, g_t)

                    # wave 2: f, o (f also accumulates the K=2 boundary term)
                    ps_f = psp.tile([128, HALF], f32, name=f"psf{s}_{h2}", tag="ps")
                    ps_o = psp.tile([128, HALF], f32, name=f"pso{s}_{h2}", tag="ps")
                    for k in range(KT):
                        rhs = a_t[:, k, bs]
                        nc.tensor.matmul(
                            ps_f, m_t[:, k, 256:384], rhs,
                            start=(k == 0), stop=False,
                        )
                        nc.tensor.matmul(
                            ps_o, m_t[:, k, 384:512], rhs,
                            start=(k == 0), stop=(k == KT - 1),
                        )
                    nc.tensor.matmul(
                        ps_f, wb_t[:, s * 128 : (s + 1) * 128], bd_t[:, bs],
                        start=False, stop=True,
                    )
                    f_t = gp.tile([128, HALF], f32, name=f"f{s}_{h2}", tag="g")
                    o_t = gp.tile([128, HALF], f32, name=f"o{s}_{h2}", tag="g")
                    nc.scalar.activation(
                        f_t, ps_f, SIG, bias=bias_t[:, b0 + 2 : b0 + 3]
                    )
                    nc.scalar.activation(
                        o_t, ps_o, SIG, bias=bias_t[:, b0 + 3 : b0 + 4]
                    )

                    # c' = f*c_prev + i*g ; h = o*tanh(c')
                    cn = outp.tile([128, HALF], f32, name=f"cn{s}_{h2}", tag="cn")
                    nc.vector.tensor_mul(cn, f_t, ct_t[:, bs])
                    nc.vector.tensor_add(cn, cn, ig_t)
                    th = ep.tile([128, HALF], f32, name=f"th{s}_{h2}", tag="th")
                    nc.scalar.activation(th, cn, TANH)
                    hn = outp.tile([128, HALF], f32, name=f"hn{s}_{h2}", tag="hn")
                    nc.vector.tensor_mul(hn, o_t, th)

                    nc.gpsimd.dma_start(
                        out=ct_o[s * 128 : (s + 1) * 128, bs], in_=cn
                    )
                    nc.gpsimd.dma_start(
                        out=ht_o[s * 128 : (s + 1) * 128, bs], in_=hn
                    )
    nc.compile()
    return nc


def _get_program():
    global _PROG
    if _PROG is None:
        _PROG = _build_program()
    return _PROG


def _prep_inputs(inputs):
    """Host-side marshalling: fused bf16 weight matrix + transposed acts."""
    f = np.float32
    x = np.asarray(inputs["x"], f)
    h_prev = np.asarray(inputs["h_prev"], f)
    c_prev = np.asarray(inputs["c_prev"], f)
    boundary = np.asarray(inputs["boundary"], f)

    gates = ["i", "g", "f", "o"]
    W = {z: np.asarray(inputs[f"W_{z}"], f) for z in gates}
    U = {z: np.asarray(inputs[f"U_{z}"], f) for z in gates}
    bias = {
        z: np.asarray(inputs[f"b_W{z}"], f) + np.asarray(inputs[f"b_U{z}"], f)
        for z in gates
    }
    W_b = np.asarray(inputs["W_b"], f)
    b_Wb = np.asarray(inputs["b_Wb"], f)
    bias["f"] = bias["f"] + b_Wb

    # M [1536, 4096]: rows 0-511 W.T, rows 512-1535 U.T; columns grouped per
    # 128-wide h-slice as [i | g | f | o].
    M = np.empty((KTOT, 4 * H), f)
    BIAS = np.empty((128, 4 * NS), f)
    for s in range(NS):
        hs = slice(s * 128, (s + 1) * 128)
        for gi, z in enumerate(gates):
            cs = slice(s * GW + gi * 128, s * GW + (gi + 1) * 128)
            M[:IN, cs] = W[z][hs].T
            M[IN:, cs] = U[z][hs].T
            BIAS[:, 4 * s + gi] = bias[z][hs]

    Mb = M.astype(BF16)
    WBT = np.ascontiguousarray(W_b.T).astype(BF16)  # [2, 1024]
    AT = np.concatenate([x, h_prev], axis=1).T  # [1536, 8192] f32
    ATb = np.ascontiguousarray(AT).astype(BF16)
    BDT = np.ascontiguousarray(boundary.T).astype(BF16)  # [2, 8192]

    in_maps = []
    for c in range(NCORES):
        rs = slice(c * BLOC, (c + 1) * BLOC)
        in_maps.append(
            {
                "a_in": np.ascontiguousarray(ATb[:, rs]),
                "m_in": Mb,
                "wb_in": WBT,
                "bd_in": np.ascontiguousarray(BDT[:, rs]),
                "bias_in": BIAS,
                "ct_in": np.ascontiguousarray(c_prev[rs].T),
            }
        )
    return in_maps


def run(inputs, trace=False):
    """Returns ((h, c), BassKernelResults)."""
    from concourse.bass_utils import run_bass_kernel_spmd

    nc = _get_program()
    in_maps = _prep_inputs(inputs)
    res = run_bass_kernel_spmd(
        nc, in_maps, core_ids=list(range(NCORES)), trace=trace
    )
    h = np.concatenate(
        [np.ascontiguousarray(r["ht_out"].T) for r in res.results], axis=0
    )
    c = np.concatenate(
        [np.ascontiguousarray(r["ct_out"].T) for r in res.results], axis=0
    )
    return (h, c), res


def kernel(**inputs):
    out, _ = run(inputs, trace=False)
    return out
